# revision 2
# baseline (speedup 1.0000x reference)
"""GATv2 layer (N=50000, D=128, H=4, E=600000) on 8 trn2 NeuronCores — v3.

Strategy: destination nodes sharded 6250/core (host-side balancing
permutation); edges partitioned by destination window (128 nodes). Per-edge
xl[src] rows fetched with batched gpsimd.dma_gather (int16 indices -> xl
table split at row 32768, per-window edges grouped lo/hi). xr[dst] per edge
is NOT gathered: it is produced by a one-hot matmul selE @ xr_win from an
SBUF-resident bf16 xr table (halves SWDGE descriptor-emission time, the
dominant cost). All matmuls run in bf16 (single-pass + fast weight load);
accumulation stays fp32 in PSUM. Scores: z = PreLU(xl+xr) (slope 0.2 via
Prelu alpha), score = sum_c z*att, alpha = exp(score) (no segment-max).
Aggregation rhs = [alpha*xl | alpha] in bf16; one-hot scatter matmul per
chunk accumulates features + denominators. BatchNorm stats AllReduced.
"""

import math
import numpy as np
import ml_dtypes

import concourse.bass as bass
import concourse.bacc as bacc
import concourse.mybir as mybir
import concourse.tile as tile
from concourse.masks import make_identity
from concourse.bass_utils import run_bass_kernel_spmd

P = 128
F32 = mybir.dt.float32
BF16 = mybir.dt.bfloat16
I16 = mybir.dt.int16
BF_NP = ml_dtypes.bfloat16

NEG_SLOPE = 0.2
BN_EPS = 1e-5
LO_ROWS = 32768          # int16 index range for the lo table
LB = 8                   # tiles per batched DMA in phases 1/4


class Cfg:
    def __init__(self, N, D, H, E, n_cores, c_lo, c_hi):
        assert D == 128
        self.N, self.D, self.H, self.E = N, D, H, E
        self.C = D // H
        self.n_cores = n_cores
        assert N % n_cores == 0
        self.NPC = N // n_cores               # nodes per core
        self.W = math.ceil(self.NPC / P)      # dst windows per core
        self.LASTR = self.NPC - P * (self.W - 1)
        self.NT = math.ceil(N / P)            # h tiles for xl table
        self.NT_LO = LO_ROWS // P             # 256 tiles -> xl_lo
        self.HI_ROWS = self.NT * P - LO_ROWS
        self.c_lo = c_lo                      # [W] lo chunks per window
        self.c_hi = c_hi                      # [W] hi chunks per window
        self.c_w = [a + b for a, b in zip(c_lo, c_hi)]
        self.K = max(self.c_w)
        assert self.K <= 16, "xre PSUM tile capped at 4 banks (16 chunks)"
        # int16 idx pack column offsets (per window: lo, hi regions)
        self.idx_cols = []
        off = 0
        for w in range(self.W):
            lo, hi = self.c_lo[w] * 8, self.c_hi[w] * 8
            self.idx_cols.append((off, off + lo, off + lo + hi))
            off += lo + hi
        self.idx_tot = off
        # dstrel / dstslotT pack column offsets
        self.rel_cols = []
        off = 0
        for w in range(self.W):
            self.rel_cols.append(off)
            off += self.c_w[w]
        self.rel_tot = off


def build_kernel(nc: bass.Bass, cfg: Cfg):
    N, H, W = cfg.N, cfg.H, cfg.W
    NT, NT_LO, LASTR = cfg.NT, cfg.NT_LO, cfg.LASTR
    DEN = P + H   # 132: aggregated features + per-head denominator columns

    # ---- I/O ----
    hfullT = nc.declare_dram_parameter("hfullT", [P, NT * P], BF16,
                                       isOutput=False)
    hlocT = nc.declare_dram_parameter("hlocT", [P, W * P], BF16,
                                      isOutput=False)
    hloc = nc.declare_dram_parameter("hloc", [W * P, P], F32, isOutput=False)
    constsA = nc.declare_dram_parameter("constsA", [P, 3], F32,
                                        isOutput=False)
    constsW = nc.declare_dram_parameter("constsW", [P, 256], BF16,
                                        isOutput=False)
    idxpack = nc.declare_dram_parameter("idxpack", [P, cfg.idx_tot], I16,
                                        isOutput=False)
    dsT = nc.declare_dram_parameter("dsT", [P, cfg.rel_tot * P], BF16,
                                    isOutput=False)
    selpk = nc.declare_dram_parameter("selpk", [P, cfg.rel_tot * P], BF16,
                                      isOutput=False)
    out = nc.declare_dram_parameter("out", [cfg.NPC, P], F32, isOutput=True)

    # ---- internal DRAM ----
    xl_lo = nc.dram_tensor("xl_lo", [LO_ROWS, P], BF16)
    xl_hi = nc.dram_tensor("xl_hi", [cfg.HI_ROWS, P], BF16)
    st_in = nc.dram_tensor("st_in", [P, 2], F32)
    st_out = nc.dram_tensor("st_out", [P, 2], F32, addr_space="Shared")

    qctr = [0]

    def next_q():
        q = qctr[0] % 4
        qctr[0] += 1
        return q

    with tile.TileContext(nc) as tc:
        import contextlib
        with contextlib.ExitStack() as ctx:
            cst = ctx.enter_context(tc.tile_pool(name="cst", bufs=1))
            sb = ctx.enter_context(tc.tile_pool(name="sb", bufs=4))
            gp = ctx.enter_context(tc.tile_pool(name="gp", bufs=6))
            psb = ctx.enter_context(tc.tile_pool(name="psb", bufs=2,
                                                 space="PSUM"))
            psw = ctx.enter_context(tc.tile_pool(name="psw", bufs=3,
                                                 space="PSUM"))
            ps1 = ctx.enter_context(tc.tile_pool(name="ps1", bufs=1,
                                                 space="PSUM"))

            # ================= constants ==============
            csA = cst.tile([P, 3], F32, tag="csA")
            nc.sync.dma_start(out=csA[:], in_=constsA[:])
            csW = cst.tile([P, 256], BF16, tag="csW")
            nc.sync.dma_start(out=csW[:], in_=constsW[:])
            ipk = cst.tile([P, cfg.idx_tot], I16, tag="ipk")
            nc.sync.dma_start(out=ipk[:], in_=idxpack[:])
            Wl_sb = csW[:, 0:P]
            Wr_sb = csW[:, P:2 * P]
            att_col = csA[:, 0:1]
            gam_col = csA[:, 1:2]
            bet_col = csA[:, 2:3]

            ones_col = cst.tile([P, 1], F32, tag="ones_c")
            nc.gpsimd.memset(ones_col[:], 1.0)
            ident = cst.tile([P, P], F32, tag="ident")
            make_identity(nc, ident[:])
            eps_col = cst.tile([P, 1], F32, tag="epsc")
            nc.gpsimd.memset(eps_col[:], BN_EPS)
            pidx = cst.tile([P, 1], mybir.dt.int32, tag="pidx")
            nc.gpsimd.iota(pidx[:], pattern=[[0, 1]], channel_multiplier=1)
            pidx_bf = cst.tile([P, 1], BF16, tag="pidx_bf")
            nc.vector.tensor_copy(pidx_bf[:], pidx[:])
            ones_msk = cst.tile([P, 1], F32, tag="ones_m")
            if LASTR < P:
                nc.vector.tensor_scalar(out=ones_msk[:], in0=pidx[:],
                                        scalar1=LASTR, scalar2=None,
                                        op0=mybir.AluOpType.is_lt)
            else:
                nc.gpsimd.memset(ones_msk[:], 1.0)

            # att replicated to all partitions: transpose of broadcast column
            att_ps = psb.tile([P, LB, P], F32, tag="pb")
            nc.tensor.transpose(att_ps[:, 0, :], att_col.to_broadcast([P, P]),
                                ident[:])
            att_rep = cst.tile([P, P], BF16, tag="attrep")
            nc.scalar.copy(att_rep[:], att_ps[:, 0, :])

            # xr table kept resident in SBUF (bf16)
            xr_res = cst.tile([P, W, P], BF16, tag="xr_res")

            # ================= phase 1: xl tables (all nodes) ================
            nbatch = math.ceil(NT / LB)
            for b in range(nbatch):
                t0 = b * LB
                nt = min(LB, NT - t0)
                hTb = sb.tile([P, LB * P], BF16, tag="hTb")
                nc.sync.dma_start(out=hTb[:, 0:nt * P],
                                  in_=hfullT[:, t0 * P:(t0 + nt) * P])
                pb = psb.tile([P, LB, P], F32, tag="pb")
                for i in range(nt):
                    nc.tensor.matmul(pb[:, i, :],
                                     lhsT=hTb[:, i * P:(i + 1) * P],
                                     rhs=Wl_sb, start=True, stop=True)
                xb = sb.tile([P, LB, P], BF16, tag="xb")
                nc.scalar.copy(xb[:, 0:nt, :], pb[:, 0:nt, :])
                r0 = t0 * P
                if t0 < NT_LO:
                    nc.sync.dma_start(
                        out=xl_lo[r0:r0 + nt * P, :]
                            .rearrange("(a p) f -> p a f", p=P),
                        in_=xb[:, 0:nt, :])
                else:
                    r0 -= LO_ROWS
                    nc.sync.dma_start(
                        out=xl_hi[r0:r0 + nt * P, :]
                            .rearrange("(a p) f -> p a f", p=P),
                        in_=xb[:, 0:nt, :])

            # ================= phase 1b: xr table -> SBUF bf16 ==============
            for b in range(math.ceil(W / LB)):
                w0 = b * LB
                nw = min(LB, W - w0)
                hTb = sb.tile([P, LB * P], BF16, tag="hTb")
                nc.sync.dma_start(out=hTb[:, 0:nw * P],
                                  in_=hlocT[:, w0 * P:(w0 + nw) * P])
                pb = psb.tile([P, LB, P], F32, tag="pb")
                for i in range(nw):
                    nc.tensor.matmul(pb[:, i, :],
                                     lhsT=hTb[:, i * P:(i + 1) * P],
                                     rhs=Wr_sb, start=True, stop=True)
                nc.scalar.copy(xr_res[:, w0:w0 + nw, :], pb[:, 0:nw, :])

            # Gate the gathers (Pool engine) on all xl-table stores having
            # landed: HWDGE DMAs drain FIFO per ring, so a probe load issued
            # after the last store completes only after every store; a pool
            # op consuming the probe orders all later pool work behind it.
            probe = cst.tile([P, 1], BF16, tag="probe")
            nc.sync.dma_start(out=probe[:], in_=xl_lo[0:P, 0:1])
            probe2 = cst.tile([P, 1], BF16, tag="probe2")
            nc.gpsimd.tensor_copy(probe2[:], probe[:])

            # ================= phase 2: edges =================
            stats_ps = ps1.tile([P, 2], F32, tag="stats")
            outpre = cst.tile([P, W, P], F32, tag="outpre")

            for w in range(W):
                c_lo, c_hi, c_w = cfg.c_lo[w], cfg.c_hi[w], cfg.c_w[w]
                o0, o1, o2 = cfg.idx_cols[w]
                r0 = cfg.rel_cols[w]

                G = gp.tile([P, cfg.K, P], BF16, tag="G")
                nc.gpsimd.dma_gather(
                    G[:, 0:c_lo, :], xl_lo[:], ipk[:, o0:o1],
                    c_lo * P, c_lo * P, P, queue_num=next_q(),
                    single_packet=False)
                if c_hi:
                    nc.gpsimd.dma_gather(
                        G[:, c_lo:c_w, :], xl_hi[:], ipk[:, o1:o2],
                        c_hi * P, c_hi * P, P, queue_num=next_q(),
                        single_packet=False)

                # selE[s, (c e)] one-hot, pre-built on host
                selE = sb.tile([P, cfg.K, P], BF16, tag="selE")
                nc.sync.dma_start(out=selE[:, 0:c_w, :],
                                  in_=dsT[:, r0 * P:(r0 + c_w) * P]
                                      .rearrange("p (a e) -> p a e", e=P))
                # xre = selE @ xr_win, in halves of <=8 chunks so the 2-bank
                # PSUM tiles double-buffer across windows
                y = sb.tile([P, cfg.K, P], BF16, tag="y")
                for h0 in range(0, c_w, LB):
                    h1 = min(h0 + LB, c_w)
                    xre = psb.tile([P, LB, P], F32, tag="pb")
                    for c in range(h0, h1):
                        nc.tensor.matmul(xre[:, c - h0, :],
                                         lhsT=selE[:, c, :],
                                         rhs=xr_res[:, w, :],
                                         start=True, stop=True)
                    nc.vector.tensor_add(y[:, h0:h1, :], G[:, h0:h1, :],
                                         xre[:, 0:h1 - h0, :])
                nc.scalar.activation(y[:, 0:c_w, :], y[:, 0:c_w, :],
                                     mybir.ActivationFunctionType.Prelu,
                                     alpha=NEG_SLOPE)
                nc.vector.tensor_mul(
                    y[:, 0:c_w, :], y[:, 0:c_w, :],
                    att_rep[:, None, :].to_broadcast([P, c_w, P]))
                s16 = sb.tile([P, cfg.K * H], F32, tag="s16")
                nc.vector.tensor_reduce(
                    out=s16[:, 0:c_w * H]
                        .rearrange("p (a h) -> p a h", h=H)[:, :, :, None],
                    in_=y[:, 0:c_w, :]
                        .rearrange("p a (h c) -> p a h c", c=cfg.C),
                    op=mybir.AluOpType.add, axis=mybir.AxisListType.X)
                rhs = sb.tile([P, cfg.K, DEN], BF16, tag="rhs")
                nc.scalar.activation(
                    rhs[:, 0:c_w, P:DEN],
                    s16[:, 0:c_w * H].rearrange("p (a h) -> p a h", h=H),
                    mybir.ActivationFunctionType.Exp)
                nc.vector.tensor_mul(
                    rhs[:, 0:c_w, 0:P]
                        .rearrange("p a (h c) -> p a h c", c=cfg.C),
                    G[:, 0:c_w, :]
                        .rearrange("p a (h c) -> p a h c", c=cfg.C),
                    rhs[:, 0:c_w, P:DEN][:, :, :, None]
                        .to_broadcast([P, c_w, H, cfg.C]))
                sel = sb.tile([P, cfg.K, P], BF16, tag="sel")
                nc.sync.dma_start(out=sel[:, 0:c_w, :],
                                  in_=selpk[:, r0 * P:(r0 + c_w) * P]
                                      .rearrange("p (a s) -> p a s", s=P))

                wps = psw.tile([P, DEN], F32, tag="wps")
                for c in range(c_w):
                    nc.tensor.matmul(
                        wps[:], lhsT=sel[:, c, :], rhs=rhs[:, c, :],
                        start=(c == 0), stop=(c == c_w - 1))

                # normalize window: out_pre = agg / max(den, tiny)
                dmx = sb.tile([P, H], F32, tag="dmx")
                nc.vector.tensor_scalar_max(dmx[:], wps[:, P:DEN], 1e-30)
                rec = sb.tile([P, H], F32, tag="rec")
                nc.vector.reciprocal(rec[:], dmx[:])
                nc.vector.tensor_mul(
                    outpre[:, w, :].rearrange("p (h c) -> p h c", c=cfg.C),
                    wps[:, 0:P].rearrange("p (h c) -> p h c", c=cfg.C),
                    rec[:, :, None].to_broadcast([P, H, cfg.C]))
                sq = sb.tile([P, P], F32, tag="sq")
                nc.scalar.square(sq[:], outpre[:, w, :])
                stat_ones = ones_msk if w == W - 1 else ones_col
                nc.tensor.matmul(stats_ps[:, 0:1], lhsT=outpre[:, w, :],
                                 rhs=stat_ones[:],
                                 start=(w == 0), stop=(w == W - 1))
                nc.tensor.matmul(stats_ps[:, 1:2], lhsT=sq[:],
                                 rhs=stat_ones[:],
                                 start=(w == 0), stop=(w == W - 1))

            # ================= phase 3: BN stats AllReduce =================
            st_sb = sb.tile([P, 2], F32, tag="stsb")
            nc.scalar.copy(st_sb[:], stats_ps[:])
            nc.sync.dma_start(out=st_in[:], in_=st_sb[:])
            tc.strict_bb_all_engine_barrier()
            nc.gpsimd.collective_compute(
                "AllReduce", mybir.AluOpType.add,
                replica_groups=[list(range(cfg.n_cores))],
                ins=[st_in[:]], outs=[st_out[:]])
            tc.strict_bb_all_engine_barrier()
            st_all = sb.tile([P, 2], F32, tag="stall")
            nc.sync.dma_start(out=st_all[:], in_=st_out[:])

            # A = gamma * rsqrt(var+eps); B = beta - A*mu  (y = A*x + B)
            mu_c = sb.tile([P, 1], F32, tag="mu")
            nc.scalar.mul(mu_c[:], st_all[:, 0:1], 1.0 / N)
            ex2 = sb.tile([P, 1], F32, tag="ex2")
            nc.scalar.mul(ex2[:], st_all[:, 1:2], 1.0 / N)
            mu2 = sb.tile([P, 1], F32, tag="mu2")
            nc.scalar.square(mu2[:], mu_c[:])
            var_c = sb.tile([P, 1], F32, tag="var")
            nc.vector.tensor_sub(var_c[:], ex2[:], mu2[:])
            sd = sb.tile([P, 1], F32, tag="sd")
            nc.scalar.activation(sd[:], var_c[:],
                                 mybir.ActivationFunctionType.Sqrt,
                                 bias=eps_col[:])
            rsd = sb.tile([P, 1], F32, tag="rsd")
            nc.vector.reciprocal(rsd[:], sd[:])
            A_c = sb.tile([P, 1], F32, tag="Ac")
            nc.vector.tensor_mul(A_c[:], gam_col, rsd[:])
            Amu = sb.tile([P, 1], F32, tag="Amu")
            nc.vector.tensor_mul(Amu[:], A_c[:], mu_c[:])
            B_c = sb.tile([P, 1], F32, tag="Bc")
            nc.vector.tensor_sub(B_c[:], bet_col, Amu[:])

            A_ps = psb.tile([P, LB, P], F32, tag="pb")
            nc.tensor.transpose(A_ps[:, 0, :], A_c[:].to_broadcast([P, P]),
                                ident[:])
            A_rep = cst.tile([P, P], F32, tag="Arep")
            nc.scalar.copy(A_rep[:], A_ps[:, 0, :])
            B_ps = psb.tile([P, LB, P], F32, tag="pb")
            nc.tensor.transpose(B_ps[:, 0, :], B_c[:].to_broadcast([P, P]),
                                ident[:])
            B_rep = cst.tile([P, P], F32, tag="Brep")
            nc.scalar.copy(B_rep[:], B_ps[:, 0, :])

            # ========== phase 4: BN apply + relu + residual (batched) =======
            for b in range(math.ceil(W / LB)):
                w0 = b * LB
                nw = min(LB, W - w0)
                full = nw * P if w0 + nw < W else (nw - 1) * P + LASTR
                t1 = sb.tile([P, LB, P], F32, tag="t1")
                nc.vector.tensor_mul(
                    t1[:, 0:nw, :], outpre[:, w0:w0 + nw, :],
                    A_rep[:, None, :].to_broadcast([P, nw, P]))
                nc.vector.tensor_add(
                    t1[:, 0:nw, :], t1[:, 0:nw, :],
                    B_rep[:, None, :].to_broadcast([P, nw, P]))
                nc.scalar.activation(t1[:, 0:nw, :], t1[:, 0:nw, :],
                                     mybir.ActivationFunctionType.Relu)
                hres = sb.tile([P, LB, P], F32, tag="hres")
                nc.sync.dma_start(
                    out=hres[:, 0:nw, :],
                    in_=hloc[w0 * P:(w0 + nw) * P, :]
                        .rearrange("(a p) f -> p a f", p=P))
                nc.vector.tensor_add(t1[:, 0:nw, :], t1[:, 0:nw, :],
                                     hres[:, 0:nw, :])
                if w0 + nw < W:
                    nc.sync.dma_start(
                        out=out[w0 * P:(w0 + nw) * P, :]
                            .rearrange("(a p) f -> p a f", p=P),
                        in_=t1[:, 0:nw, :])
                else:
                    if nw > 1:
                        nc.sync.dma_start(
                            out=out[w0 * P:(w0 + nw - 1) * P, :]
                                .rearrange("(a p) f -> p a f", p=P),
                            in_=t1[:, 0:nw - 1, :])
                    nc.sync.dma_start(
                        out=out[(w0 + nw - 1) * P:(w0 + nw - 1) * P + LASTR,
                                :],
                        in_=t1[:LASTR, nw - 1, :])
    return nc


def _wrap16(idx, ncols):
    """[n] ints (n = ncols*16) -> [128, ncols] int16 wrapped+replicated."""
    a = np.asarray(idx, np.int16).reshape(-1, 16).T
    return np.tile(a, (8, 1))


def host_prepare(h, edge_index, W_l, W_r, bias_l, bias_r, att,
                 bias_out, gamma, beta, n_cores=8):
    """Balance nodes into windows, shard edges by destination window, build
    int16 gather-index packs. Returns (cfg, in_maps, perm)."""
    import heapq

    N, D = h.shape
    H, C = att.shape
    E = edge_index.shape[1]
    h = np.asarray(h, np.float32)
    ei = np.asarray(edge_index)

    loops = np.arange(N, dtype=np.int64)
    src = np.concatenate([ei[0], loops]).astype(np.int32)
    dst = np.concatenate([ei[1], loops]).astype(np.int32)

    NPC = N // n_cores
    W = math.ceil(NPC / P)
    NB = n_cores * W
    LASTR = NPC - P * (W - 1)

    lo_mask = src < LO_ROWS
    deg_lo = np.bincount(dst[lo_mask], minlength=N)
    deg_hi = np.bincount(dst[~lo_mask], minlength=N)

    caps = np.full(NB, P, np.int32)
    caps[NB - n_cores:] = LASTR
    order = np.argsort(-(deg_lo.astype(np.int64)))
    heap = [(0, 0, b) for b in range(NB)]
    heapq.heapify(heap)
    bin_nodes = [[] for _ in range(NB)]
    bin_lo = np.zeros(NB, np.int64)
    bin_hi = np.zeros(NB, np.int64)
    for nid in order:
        while True:
            nlo, nhi, b = heapq.heappop(heap)
            if len(bin_nodes[b]) < caps[b]:
                break
        bin_nodes[b].append(nid)
        bin_lo[b] += deg_lo[nid]
        bin_hi[b] += deg_hi[nid]
        if len(bin_nodes[b]) < caps[b]:
            heapq.heappush(heap, (int(bin_lo[b]), int(bin_hi[b]), b))

    full_bins = [b for b in range(NB) if caps[b] == P]
    small_bins = [b for b in range(NB) if caps[b] == LASTR]
    full_bins.sort(key=lambda b: -bin_lo[b])
    slot = [[None] * W for _ in range(n_cores)]
    for i, b in enumerate(full_bins):
        w, k = divmod(i, n_cores)
        slot[k][w] = b
    for k, b in enumerate(small_bins):
        slot[k][W - 1] = b

    perm = np.empty(N, np.int64)
    for k in range(n_cores):
        pos = k * NPC
        for w in range(W):
            nodes = bin_nodes[slot[k][w]]
            perm[pos:pos + len(nodes)] = nodes
            pos += len(nodes)
        assert pos == (k + 1) * NPC

    node_core = np.empty(N, np.int32)
    node_win = np.empty(N, np.int32)
    node_slot = np.empty(N, np.int32)
    for k in range(n_cores):
        for w in range(W):
            nodes = np.asarray(bin_nodes[slot[k][w]], np.int64)
            node_core[nodes] = k
            node_win[nodes] = w
            node_slot[nodes] = np.arange(len(nodes))

    e_core = node_core[dst]
    e_win = node_win[dst]
    e_lo = (src < LO_ROWS)
    key = (e_core.astype(np.int64) * W + e_win) * 2 + (~e_lo)
    eorder = np.argsort(key, kind="stable")
    src_s = src[eorder]
    dst_s = dst[eorder]
    key_s = key[eorder]
    bounds = np.searchsorted(key_s, np.arange(NB * 2 + 1))

    cnt_lo = (bounds[1::2] - bounds[0:-1:2]).reshape(n_cores, W)
    cnt_hi = (bounds[2::2] - bounds[1::2]).reshape(n_cores, W)
    c_lo = [max(int(math.ceil(cnt_lo[:, w].max() / P)), 1) for w in range(W)]
    c_hi = [int(math.ceil(cnt_hi[:, w].max() / P)) for w in range(W)]

    cfg = Cfg(N=N, D=D, H=H, E=E + N, n_cores=n_cores, c_lo=c_lo, c_hi=c_hi)

    constsA = np.zeros((P, 3), np.float32)
    constsA[:, 0] = np.asarray(att, np.float32).reshape(-1)
    constsA[:, 1] = np.asarray(gamma, np.float32)
    constsA[:, 2] = np.asarray(beta, np.float32)
    constsW = np.zeros((P, 256), BF_NP)
    constsW[:, 0:P] = np.asarray(W_l, np.float32).astype(BF_NP)
    constsW[:, P:2 * P] = np.asarray(W_r, np.float32).astype(BF_NP)

    hfull = np.zeros((cfg.NT * P, P), np.float32)
    hfull[:N] = h
    hfullT = np.ascontiguousarray(hfull.T).astype(BF_NP)

    in_maps = []
    for k in range(n_cores):
        ipk = np.zeros((P, cfg.idx_tot), np.int16)
        dsE = np.zeros((P, cfg.rel_tot * P), BF_NP)
        spk = np.zeros((P, cfg.rel_tot * P), BF_NP)
        for w in range(W):
            o0, o1, o2 = cfg.idx_cols[w]
            r0 = cfg.rel_cols[w]
            klo, khi, kw = cfg.c_lo[w], cfg.c_hi[w], cfg.c_w[w]
            b_lo0 = bounds[(k * W + w) * 2]
            b_lo1 = bounds[(k * W + w) * 2 + 1]
            b_hi1 = bounds[(k * W + w) * 2 + 2]
            s_lo = src_s[b_lo0:b_lo1]
            d_lo = dst_s[b_lo0:b_lo1]
            s_hi = src_s[b_lo1:b_hi1]
            d_hi = dst_s[b_lo1:b_hi1]
            # sort each group by src so gather descriptors walk HBM forward
            so = np.argsort(s_lo, kind="stable")
            s_lo, d_lo = s_lo[so], d_lo[so]
            so = np.argsort(s_hi, kind="stable")
            s_hi, d_hi = s_hi[so], d_hi[so]
            buf = np.zeros(klo * P, np.int32)
            buf[:len(s_lo)] = s_lo
            ipk[:, o0:o1] = _wrap16(buf, klo * 8)
            if khi:
                buf = np.zeros(khi * P, np.int32)
                buf[:len(s_hi)] = s_hi - LO_ROWS
                ipk[:, o1:o2] = _wrap16(buf, khi * 8)
            rel = np.full(kw * P, -1, np.int64)
            rel[:len(d_lo)] = node_slot[d_lo]
            rel[klo * P:klo * P + len(d_hi)] = node_slot[d_hi]
            pos = np.nonzero(rel >= 0)[0]
            slots = rel[pos]
            # selE[s, (c,e)]: one-hot down partitions
            dsE[slots, r0 * P + pos] = 1
            # sel[e, (c,s)]: partition = e = pos%128, col = c*128 + slot
            spk[pos % P, r0 * P + (pos // P) * P + slots] = 1

        hl = np.zeros((W * P, P), np.float32)
        hl[:NPC] = h[perm[k * NPC:(k + 1) * NPC]]
        in_maps.append({
            "hfullT": hfullT,
            "hlocT": np.ascontiguousarray(hl.T).astype(BF_NP), "hloc": hl,
            "constsA": constsA, "constsW": constsW,
            "idxpack": ipk, "dsT": dsE, "selpk": spk,
        })
    return cfg, in_maps, perm


def kernel(h, edge_index, W_l, W_r, bias_l, bias_r, att,
           bias_out, gamma, beta):
    n_cores = 8
    cfg, in_maps, perm = host_prepare(
        h, edge_index, W_l, W_r, bias_l, bias_r, att, bias_out, gamma, beta,
        n_cores=n_cores)
    nc = bacc.Bacc(num_swdge_queues=4)
    build_kernel(nc, cfg)
    nc.compile()
    res = run_bass_kernel_spmd(nc, in_maps, core_ids=list(range(n_cores)))
    outs = np.concatenate([res.results[k]["out"] for k in range(n_cores)],
                          axis=0)
    full = np.empty_like(outs)
    full[perm] = outs
    return full.astype(np.float32)
